# revision 43
# baseline (speedup 1.0000x reference)
"""nn_BinaryConv2D Trainium2 kernel.

out = conv2d(sign(x), sign(w)), 3x3, stride 1, SAME, NHWC/HWIO.
x [64, 128, 128, 64] fp32, w [3, 3, 64, 64] fp32 -> out [64, 128, 128, 64] fp32.

Sharding: data-parallel over batch across 8 NeuronCores (8 images/core);
the tiny weight is sign()ed host-side, packed into per-pass tap stacks,
and replicated to every core.

Per-core scheme (all shapes hardcoded):
- Each image is loaded as one fully-linear [128, 8192] bf16 tile (row
  per partition, 32 KB contiguous DRAM reads) through a SWDGE DMA that
  casts fp32 -> bf16 in flight (sign is preserved exactly by the cast;
  the cost model charges DMA on output bytes, halving input traffic).
- The pixel-major -> channel-major transpose runs on the DMA xbar: a
  single 63-block dma_start_transpose per image over the contiguous
  column slice starting at pixel 1, so block g already holds the
  pair-skewed stack [odd@g ci | even@(g+1) ci] x 128 rows.  One xbar
  per image matters: the Tile scheduler serializes every DMA transpose
  against all in-flight DMAs (deadlock workaround), so each xbar is a
  DMA barrier.  The two edge columns (pixel 127's odd@63 and pixel 0's
  even@0) are transposed on the PE via bf16 identity matmuls.
- One full-width ACT Sign per image (plus two tiny edge Signs) writes
  bf16 -> mega fp8e4 with hand-built 3D APs: mega column (row slot r,
  pair j) holds [top: odd@j | bottom: even@(j+1)], +-1, with 0 pads
  (memsets cover the half-pad columns: top of col 0, bottom of col 64).
- Conv = 6 fp8e4 K=128 matmuls per PSUM group of 6 row-slots (N=390):
  per dy one aligned pass (odd@j dense for both output parities +
  even@(j+1) -> odd out) and one -1-shifted pass (odd@(j-1) -> even
  out + even@j dense).  6 passes is the non-DoubleRow floor (12
  distinct tap streams, 2 per pass); DoubleRow would halve this in the
  cost model but faults this container's runtime in any configuration.
- DVE strips pad columns and casts PSUM fp32 -> fp16 (exact: outputs
  are integers in [-576, 576]).
- Output is stored channel-major ([img, (parity, co), row, pair]) as
  fp16 on the SP HWDGE queue and unshuffled to NHWC fp32 on the host -
  no output transpose.
- Emission is software-pipelined: image loads are prefetched one
  iteration ahead; each iteration runs the edge transposes, the xbar +
  Sign chain of image i, and the conv batches of image i-1.
"""

from contextlib import ExitStack

import numpy as np
import ml_dtypes

import concourse.bass as bass
import concourse.tile as tile
from concourse import mybir
from concourse.vector_clock import ScopedClock, VectorClock
from concourse.tile_rust import add_dep_helper

H = W = 128
C = 64
SW = 65
OFF = 1
MEGA_COLS = OFF + (H + 2) * SW + 1  # 8452
N_CORES = 8
NIMG = 8  # images per core
RC = 32  # rows per load chunk


# ---------------------------------------------------------------------------
# Workaround for this container's walrus: CTRL instructions support only ONE
# sync-wait slot, but Tile's tail drain attaches one wait per live proc.
# Split the waits across single-wait NoOps on the SP engine (in-order), then
# drain waitless.
def _drain_and_barrier_split(self, tick_clock, wait_clock):
    nc = self.nc
    vc = tick_clock.global_clock
    n = len(vc)
    for i in range(n):
        if vc[i] > 0:
            sub = VectorClock([0] * n)
            sub.require_at_least(i, vc[i])
            nop = nc.sync.nop(nofuse=True)
            wait_clock.add_sem_waits(nop.ins, ScopedClock({None: sub}))
    nc.sync.drain()
    nc.all_engine_barrier()
    assert self.sems is not None
    popped = nc._tile_sem_poison_stack.pop()
    assert popped is self._sem_poison
    nc.clear_and_free_semaphores(list(self.sems.allocated().values()))
    nc.all_engine_barrier()


tile.TileContext._drain_and_barrier = _drain_and_barrier_split


# The same walrus limit applies to every instruction: at most one sync wait.
# Tile freely emits multi-wait instructions, so rewrite the BIR JSON right
# before compilation: hoist all but the last wait of each instruction onto
# fresh same-engine NoOps inserted immediately before it (engines execute
# their instruction stream in order, so the waits still gate the original
# instruction).
def _split_multi_waits_json(bir_bytes):
    import json as _json

    bir = _json.loads(bir_bytes)
    n = 0
    for fn in bir.get("functions", []):
        for blk in fn.get("blocks", []):
            insts = blk.get("instructions", [])
            out = []
            for ins in insts:
                si = ins.get("sync_info")
                if si:
                    waits = si.get("on_wait") or []
                    if len(waits) > 1:
                        for wv in waits[:-1]:
                            n += 1
                            out.append(
                                {
                                    "debug": ins.get("debug", 0),
                                    "engine": ins["engine"],
                                    "ins": [],
                                    "outs": [],
                                    "name": f"I-wsplit-{n}",
                                    "opcode": "NoOp",
                                    "sync_info": {
                                        "on_update": [],
                                        "on_wait": [wv],
                                    },
                                }
                            )
                        si["on_wait"] = [waits[-1]]
                out.append(ins)
            blk["instructions"] = out
    return _json.dumps(bir).encode()


def _install_compile_hook():
    from concourse import bass_utils as _bu
    from concourse import bass2jax as _b2j

    if getattr(_bu, "_orig_compile_bir_kernel", None) is None:
        _bu._orig_compile_bir_kernel = _bu.compile_bir_kernel

        def _patched(bir_json, tmpdir, neff_name="file.neff"):
            return _bu._orig_compile_bir_kernel(
                _split_multi_waits_json(bir_json), tmpdir, neff_name=neff_name
            )

        _bu.compile_bir_kernel = _patched
        _b2j.compile_bir_kernel = _patched


_install_compile_hook()
# ---------------------------------------------------------------------------


def build_nc(nimg=NIMG, gsize=6, mega_bufs=2, psum_bufs=5, io_bufs=2,
             pin_bufs=2, tmp_bufs=2, cm_bufs=4, gb=6):
    nc = bass.Bass()
    x = nc.dram_tensor("x", [nimg, H, W, C], mybir.dt.float32, kind="ExternalInput")
    wt = nc.dram_tensor("wt", [128, 768], mybir.dt.float8e4, kind="ExternalInput")
    idents = nc.dram_tensor(
        "idents", [128, 256], mybir.dt.bfloat16, kind="ExternalInput"
    )
    y = nc.dram_tensor("y", [nimg, 128, H * C], mybir.dt.float16, kind="ExternalOutput")

    with tile.TileContext(nc) as tc, ExitStack() as ctx:
        wpool = ctx.enter_context(tc.tile_pool(name="wpool", bufs=1))
        mega_pool = ctx.enter_context(tc.tile_pool(name="mega", bufs=mega_bufs))
        in_pool = ctx.enter_context(tc.tile_pool(name="inp", bufs=io_bufs))
        tmp_pool = ctx.enter_context(tc.tile_pool(name="tmp", bufs=tmp_bufs))
        psum_pool = ctx.enter_context(
            tc.tile_pool(name="ps", bufs=psum_bufs, space="PSUM")
        )
        pin_pool = ctx.enter_context(
            tc.tile_pool(name="pin", bufs=pin_bufs, space="PSUM")
        )
        cm_pool = ctx.enter_context(tc.tile_pool(name="cm", bufs=cm_bufs))

        wt_sb = wpool.tile([128, 768], mybir.dt.float8e4)
        nc.sync.dma_start(out=wt_sb[:], in_=wt[:])
        id_sb = wpool.tile([128, 256], mybir.dt.bfloat16)
        nc.sync.dma_start(out=id_sb[:], in_=idents[:])

        groups = []
        r0 = 0
        while r0 < H:
            g = min(gsize, H - r0)
            groups.append((r0, g))
            r0 += g

        def input_loads(img, first_half_only=False):
            mega = mega_pool.tile([128, MEGA_COLS], mybir.dt.float8e4)

            # whole-image linear load (row per partition) with fp32 -> bf16
            # cast in the DMA (SWDGE)
            xt = in_pool.tile([128, H * 64], mybir.dt.bfloat16)
            rows = 64 if first_half_only else H
            nc.gpsimd.dma_start(
                out=xt[0:rows, :],
                in_=x[img, 0:rows].rearrange("r w c -> r (w c)"),
            )

            # zero pads.  Each slot's col 0 is half-pad: top (odd) is the
            # left-edge zero, bottom holds real data (even@0, written by the
            # ACT below).  Col 64's bottom is the right-edge zero (even@64);
            # its top holds odd@63.  Row slots -1/128 are fully zero.
            nc.gpsimd.memset(mega[:, 0:2], 0.0)
            slots = mega[:, OFF : OFF + 130 * SW].rearrange(
                "p (s c) -> p s c", c=SW
            )
            nc.gpsimd.memset(slots[0:64, :, 0:1], 0.0)
            nc.gpsimd.memset(slots[64:128, :, 64:65], 0.0)
            nc.gpsimd.memset(mega[:, OFF : OFF + SW], 0.0)
            b128 = OFF + 129 * SW
            nc.gpsimd.memset(mega[:, b128 : b128 + SW], 0.0)

            return mega, xt

        def input_edges(img, mega, xt):
            # Edge columns on the PE: odd@63 (pixel 127) and even@0
            # (pixel 0) channel transposes.
            pin = pin_pool.tile([128, 128], mybir.dt.float32)
            nc.tensor.matmul(
                pin[0:64, :],
                xt[:, 127 * 64 : 128 * 64],
                id_sb[:, 0:128],
                start=True,
                stop=True,
            )
            nc.tensor.matmul(
                pin[64:128, :],
                xt[:, 0:64],
                id_sb[:, 0:128],
                start=True,
                stop=True,
            )
            # odd@63 -> top of col 64 of each row slot
            dsl2 = mega[0:64, OFF + SW + 64 : OFF + SW + 65]
            dst2 = bass.AP(
                dsl2.tensor, dsl2.offset, [list(dsl2.ap[0]), [SW, H]]
            )
            ssl2 = pin[0:64, 0:1]
            srcap2 = bass.AP(
                ssl2.tensor, ssl2.offset, [list(ssl2.ap[0]), [1, H]]
            )
            nc.scalar.activation(
                dst2, srcap2, mybir.ActivationFunctionType.Sign
            )
            # even@0 -> bottom of col 0 of each row slot
            dsl3 = mega[64:128, OFF + SW : OFF + SW + 1]
            dst3 = bass.AP(
                dsl3.tensor, dsl3.offset, [list(dsl3.ap[0]), [SW, H]]
            )
            ssl3 = pin[64:128, 0:1]
            srcap3 = bass.AP(
                ssl3.tensor, ssl3.offset, [list(ssl3.ap[0]), [1, H]]
            )
            nc.scalar.activation(
                dst3, srcap3, mybir.ActivationFunctionType.Sign
            )

        def input_xbar_sign_full(img, mega, xt):
            # whole-image variant: a single xbar (one DMA barrier) + one
            # full-width Sign covering all 128 rows.
            tmp = tmp_pool.tile([128, 63 * 128], mybir.dt.bfloat16)
            nc.scalar.dma_start_transpose(
                out=tmp[:].rearrange("p (g f) -> p g f", f=128),
                in_=xt[:, 64 : 64 + 63 * 128],
            )
            dsl = mega[:, OFF + SW + 1 : OFF + SW + 2]
            dst = bass.AP(
                dsl.tensor,
                dsl.offset,
                [list(dsl.ap[0]), [SW, H], [1, 63]],
            )
            ssl = tmp[:, 0:1]
            srcap = bass.AP(
                ssl.tensor,
                ssl.offset,
                [list(ssl.ap[0]), [1, H], [128, 63]],
            )
            nc.scalar.activation(
                dst, srcap, mybir.ActivationFunctionType.Sign
            )

        def input_xbar_sign_half(img, mega, xt, h):
            # 64-row half variant (image 0 warm-up): two smaller xbar +
            # Sign chains let the first conv batches start ~14us earlier.
            HH = 64
            tmp = tmp_pool.tile([128, 63 * HH], mybir.dt.bfloat16)
            nc.scalar.dma_start_transpose(
                out=tmp[:].rearrange("p (g f) -> p g f", f=HH),
                in_=xt[HH * h : HH * h + HH, 64 : 64 + 63 * 128],
            )
            b0 = OFF + (HH * h + 1) * SW + 1
            dsl = mega[:, b0 : b0 + 1]
            dst = bass.AP(
                dsl.tensor,
                dsl.offset,
                [list(dsl.ap[0]), [SW, HH], [1, 63]],
            )
            ssl = tmp[:, 0:1]
            srcap = bass.AP(
                ssl.tensor,
                ssl.offset,
                [list(ssl.ap[0]), [1, HH], [HH, 63]],
            )
            nc.scalar.activation(
                dst, srcap, mybir.ActivationFunctionType.Sign
            )

        def input_warmup_pe(img, mega, xt, t):
            # warm-up path (image 0 only): transpose a 32-row chunk on the
            # otherwise-idle PE via K=32 identity matmuls, 16 pair-blocks
            # per PSUM tile, so the first conv can start ~10us in without
            # waiting for the whole-image xbar chain.
            b0 = OFF + (RC * t + 1) * SW + 1
            for q in range(4):
                pin = pin_pool.tile(
                    [128, 16 * RC], mybir.dt.float32, padded_shape=[128, 512]
                )
                for gg in range(16):
                    g = 16 * q + gg
                    if g >= 63:
                        break
                    nc.tensor.matmul(
                        pin[:, RC * gg : RC * gg + RC],
                        xt[RC * t : RC * t + RC, 64 + 128 * g : 192 + 128 * g],
                        id_sb[RC * t : RC * t + RC, RC * t : RC * t + RC],
                        start=True,
                        stop=True,
                        tile_position=(RC * t, 0),
                    )
                ng = min(16, 63 - 16 * q)
                dsl = mega[:, b0 + 16 * q : b0 + 16 * q + 1]
                dst = bass.AP(
                    dsl.tensor,
                    dsl.offset,
                    [list(dsl.ap[0]), [SW, RC], [1, ng]],
                )
                ssl = pin[:, 0:1]
                srcap = bass.AP(
                    ssl.tensor,
                    ssl.offset,
                    [list(ssl.ap[0]), [1, RC], [RC, ng]],
                )
                nc.scalar.activation(
                    dst, srcap, mybir.ActivationFunctionType.Sign
                )

        GB = gb
        batches = [groups[i : i + GB] for i in range(0, len(groups), GB)]
        if len(batches) >= 2 and len(batches[-1]) < GB // 2:
            batches[-2].extend(batches.pop())

        def compute_batch(img, mega, batch):
            if True:
                rb0 = batch[0][0]
                RB = sum(g for _, g in batch)
                cm = cm_pool.tile([128, RB * 64], mybir.dt.float16)
                cmoff = 0
                for r0, g in batch:
                    N = g * SW
                    ps = psum_pool.tile(
                        [128, N], mybir.dt.float32, padded_shape=[128, 512]
                    )
                    # 6 passes: per dy, aligned (A) + minus-one-shifted (B)
                    mms = []
                    for dy in range(3):
                        ibA = OFF + (r0 + dy) * SW
                        mms.append(nc.tensor.matmul(
                            ps[:, :],
                            wt_sb[:, dy * 256 : dy * 256 + 128],
                            mega[:, ibA : ibA + N],
                            start=(dy == 0),
                            stop=False,
                        ))
                        mms.append(nc.tensor.matmul(
                            ps[:, :],
                            wt_sb[:, dy * 256 + 128 : dy * 256 + 256],
                            mega[:, ibA - 1 : ibA - 1 + N],
                            start=False,
                            stop=(dy == 2),
                        ))
                    for a, b in zip(mms[1:], mms[:-1]):
                        add_dep_helper(
                            a.ins, b.ins, sync=False, reason="psum group order"
                        )

                    # strip pads + cast into the batch's cm tile
                    ps_v = ps[:].rearrange("p (s c) -> p s c", c=SW)[:, :, 1:65]
                    nc.vector.tensor_copy(
                        out=cm[:, cmoff : cmoff + g * 64].rearrange(
                            "p (s c) -> p s c", c=64
                        ),
                        in_=ps_v,
                    )
                    cmoff += g * 64

                # direct channel-major store on the SP HWDGE queue
                # (host unshuffles to NHWC)
                nc.sync.dma_start(
                    out=y[img, :, rb0 * 64 : (rb0 + RB) * 64], in_=cm[:]
                )

        # software-pipelined emission.  The image load is prefetched one
        # iteration ahead (emitted right after the input stage so it lands
        # on the DMA device before the stores); each iteration runs the
        # (tiny) PE edge transposes, the xbar + Sign chain for image it,
        # and the conv batches of image it-1.  Image 0 uses two half-image
        # xbars so the first conv batches start earlier.
        megas = {}
        megas[0] = input_loads(0)
        for it in range(nimg + 1):
            if it < nimg:
                mega, xt = megas[it]
                if it == 0:
                    # warm-up: two half-image xbars let the first conv
                    # batches start earlier
                    input_xbar_sign_half(it, mega, xt, 0)
                    input_xbar_sign_half(it, mega, xt, 1)
                    input_edges(it, mega, xt)
                else:
                    input_edges(it, mega, xt)
                    input_xbar_sign_full(it, mega, xt)
            if it + 1 < nimg:
                megas[it + 1] = input_loads(it + 1)
            for batch in batches:
                if it >= 1:
                    compute_batch(it - 1, megas[it - 1][0], batch)
            if it >= 1:
                del megas[it - 1]

    return nc


def make_wt(w_np):
    """Host-side weight prep: w [3,3,64,64] fp32 -> wt [128, 768] fp8e4.

    Mega column c holds [top k<64: odd@c | bottom k>=64: even@(c+1)].
    Per dy, pass A ([128,128] at cols dy*256, rhs base ibA): column j
    delivers odd@j (top) and even@(j+1) (bottom).  Pass B (at cols
    dy*256+128, rhs base ibA-1): column j delivers odd@(j-1) and even@j.
    M cols 0:64 = even-pixel outputs, 64:128 = odd-pixel outputs.
    """
    ws = np.sign(w_np).astype(np.float32)
    wt = np.zeros((128, 768), np.float32)
    for dy in range(3):
        WA = np.zeros((128, 128), np.float32)
        WA[0:64, 0:64] = ws[dy, 2]      # odd@j -> even-out (dx=2)
        WA[0:64, 64:128] = ws[dy, 1]    # odd@j -> odd-out (dx=1)
        WA[64:128, 64:128] = ws[dy, 2]  # even@(j+1) -> odd-out (dx=2)
        WB = np.zeros((128, 128), np.float32)
        WB[0:64, 0:64] = ws[dy, 0]      # odd@(j-1) -> even-out (dx=0)
        WB[64:128, 0:64] = ws[dy, 1]    # even@j -> even-out (dx=1)
        WB[64:128, 64:128] = ws[dy, 0]  # even@j -> odd-out (dx=0)
        wt[:, dy * 256 : dy * 256 + 128] = WA
        wt[:, dy * 256 + 128 : dy * 256 + 256] = WB
    fp8 = mybir.dt.np(mybir.dt.float8e4)
    return wt.astype(fp8)


def make_idents():
    """[I | Iz] [128, 256] bf16: Iz[k, n] = (k == n+1) & (n % 4 != 3)."""
    idents = np.zeros((128, 256), np.float32)
    idents[:, 0:128] = np.eye(128)
    for n in range(127):
        if n % 4 != 3:
            idents[n + 1, 128 + n] = 1.0
    return idents.astype(ml_dtypes.bfloat16)


_NC_CACHE = {}


def get_nc():
    if "nc" not in _NC_CACHE:
        _NC_CACHE["nc"] = build_nc()
    return _NC_CACHE["nc"]


def kernel(x, w):
    from concourse.bass_utils import run_bass_kernel_spmd

    x = np.asarray(x, dtype=np.float32)
    w = np.asarray(w, dtype=np.float32)
    assert x.shape == (N_CORES * NIMG, H, W, C) and w.shape == (3, 3, C, C)

    wt = make_wt(w)
    idents = make_idents()
    nc = get_nc()
    in_maps = [
        {
            "x": np.ascontiguousarray(x[c * NIMG : (c + 1) * NIMG]),
            "wt": wt,
            "idents": idents,
        }
        for c in range(N_CORES)
    ]
    res = run_bass_kernel_spmd(nc, in_maps, list(range(N_CORES)))
    outs = []
    for c in range(N_CORES):
        ycm = np.asarray(res.results[c]["y"])  # [nimg, 128, H*64] fp16
        o = (
            ycm.reshape(NIMG, 2, 64, H, 64)  # (img, par, co, r, j)
            .transpose(0, 3, 4, 1, 2)        # (img, r, j, par, co)
            .reshape(NIMG, H, W, C)
            .astype(np.float32)
        )
        outs.append(o)
    return np.concatenate(outs, axis=0)


# revision 48
# speedup vs baseline: 1.0494x; 1.0494x over previous
"""nn_BinaryConv2D Trainium2 kernel.

out = conv2d(sign(x), sign(w)), 3x3, stride 1, SAME, NHWC/HWIO.
x [64, 128, 128, 64] fp32, w [3, 3, 64, 64] fp32 -> out [64, 128, 128, 64] fp32.

Sharding: data-parallel over batch across 8 NeuronCores (8 images/core);
the tiny weight is sign()ed host-side, packed into per-pass tap stacks,
and replicated to every core.

Per-core scheme (all shapes hardcoded):
- Each image is loaded as one fully-linear [128, 8192] bf16 tile (row
  per partition, 32 KB contiguous DRAM reads) through a SWDGE DMA that
  casts fp32 -> bf16 in flight (sign is preserved exactly by the cast;
  the cost model charges DMA on output bytes, halving input traffic).
- The pixel-major -> channel-major transpose runs on the DMA xbar: a
  single 63-block dma_start_transpose per image over the contiguous
  column slice starting at pixel 1, so block g already holds the
  pair-skewed stack [odd@g ci | even@(g+1) ci] x 128 rows.  One xbar
  per image matters: the Tile scheduler serializes every DMA transpose
  against all in-flight DMAs (deadlock workaround), so each xbar is a
  DMA barrier.  The two edge columns (pixel 127's odd@63 and pixel 0's
  even@0) are transposed on the PE via bf16 identity matmuls.
- One full-width ACT Sign per image (plus two tiny edge Signs) writes
  bf16 -> mega fp8e4 with hand-built 3D APs: mega column (row slot r,
  pair j) holds [top: odd@j | bottom: even@(j+1)], +-1, with 0 pads
  (memsets cover the half-pad columns: top of col 0, bottom of col 64).
- Conv = 6 fp8e4 K=128 matmuls per PSUM group of 6 row-slots (N=390):
  per dy one aligned pass (odd@j dense for both output parities +
  even@(j+1) -> odd out) and one -1-shifted pass (odd@(j-1) -> even
  out + even@j dense).  6 passes is the non-DoubleRow floor (12
  distinct tap streams, 2 per pass); DoubleRow would halve this in the
  cost model but faults this container's runtime in any configuration.
- DVE strips pad columns and casts PSUM fp32 -> fp16 (exact: outputs
  are integers in [-576, 576]).
- Output is stored channel-major ([img, (parity, co), row, pair]) as
  fp16 on the SP HWDGE queue and unshuffled to NHWC fp32 on the host -
  no output transpose.
- Emission is software-pipelined: image loads are prefetched one
  iteration ahead; each iteration runs the edge transposes, the xbar +
  Sign chain of image i, and the conv batches of image i-1.
"""

from contextlib import ExitStack

import numpy as np
import ml_dtypes

import concourse.bass as bass
import concourse.tile as tile
from concourse import mybir
from concourse.vector_clock import ScopedClock, VectorClock
from concourse.tile_rust import add_dep_helper

H = W = 128
C = 64
SW = 65
OFF = 1
MEGA_COLS = OFF + (H + 2) * SW + 1  # 8452
N_CORES = 8
NIMG = 8  # images per core
RC = 32  # rows per load chunk


# ---------------------------------------------------------------------------
# Workaround for this container's walrus: CTRL instructions support only ONE
# sync-wait slot, but Tile's tail drain attaches one wait per live proc.
# Split the waits across single-wait NoOps on the SP engine (in-order), then
# drain waitless.
def _drain_and_barrier_split(self, tick_clock, wait_clock):
    nc = self.nc
    vc = tick_clock.global_clock
    n = len(vc)
    for i in range(n):
        if vc[i] > 0:
            sub = VectorClock([0] * n)
            sub.require_at_least(i, vc[i])
            nop = nc.sync.nop(nofuse=True)
            wait_clock.add_sem_waits(nop.ins, ScopedClock({None: sub}))
    nc.sync.drain()
    nc.all_engine_barrier()
    assert self.sems is not None
    popped = nc._tile_sem_poison_stack.pop()
    assert popped is self._sem_poison
    nc.clear_and_free_semaphores(list(self.sems.allocated().values()))
    nc.all_engine_barrier()


tile.TileContext._drain_and_barrier = _drain_and_barrier_split


# The same walrus limit applies to every instruction: at most one sync wait.
# Tile freely emits multi-wait instructions, so rewrite the BIR JSON right
# before compilation: hoist all but the last wait of each instruction onto
# fresh same-engine NoOps inserted immediately before it (engines execute
# their instruction stream in order, so the waits still gate the original
# instruction).
def _split_multi_waits_json(bir_bytes):
    import json as _json

    bir = _json.loads(bir_bytes)
    n = 0
    for fn in bir.get("functions", []):
        for blk in fn.get("blocks", []):
            insts = blk.get("instructions", [])
            out = []
            for ins in insts:
                si = ins.get("sync_info")
                if si:
                    waits = si.get("on_wait") or []
                    if len(waits) > 1:
                        for wv in waits[:-1]:
                            n += 1
                            out.append(
                                {
                                    "debug": ins.get("debug", 0),
                                    "engine": ins["engine"],
                                    "ins": [],
                                    "outs": [],
                                    "name": f"I-wsplit-{n}",
                                    "opcode": "NoOp",
                                    "sync_info": {
                                        "on_update": [],
                                        "on_wait": [wv],
                                    },
                                }
                            )
                        si["on_wait"] = [waits[-1]]
                out.append(ins)
            blk["instructions"] = out
    return _json.dumps(bir).encode()


def _install_compile_hook():
    from concourse import bass_utils as _bu
    from concourse import bass2jax as _b2j

    if getattr(_bu, "_orig_compile_bir_kernel", None) is None:
        _bu._orig_compile_bir_kernel = _bu.compile_bir_kernel

        def _patched(bir_json, tmpdir, neff_name="file.neff"):
            return _bu._orig_compile_bir_kernel(
                _split_multi_waits_json(bir_json), tmpdir, neff_name=neff_name
            )

        _bu.compile_bir_kernel = _patched
        _b2j.compile_bir_kernel = _patched


_install_compile_hook()
# ---------------------------------------------------------------------------


def build_nc(nimg=NIMG, gsize=6, mega_bufs=2, psum_bufs=5, io_bufs=2,
             pin_bufs=2, tmp_bufs=2, cm_bufs=5, gb=6):
    nc = bass.Bass()
    x = nc.dram_tensor("x", [nimg, H, W, C], mybir.dt.float32, kind="ExternalInput")
    wt = nc.dram_tensor("wt", [128, 768], mybir.dt.float8e4, kind="ExternalInput")
    idents = nc.dram_tensor(
        "idents", [128, 256], mybir.dt.bfloat16, kind="ExternalInput"
    )
    y = nc.dram_tensor("y", [nimg, 128, H * C], mybir.dt.float16, kind="ExternalOutput")

    with tile.TileContext(nc) as tc, ExitStack() as ctx:
        wpool = ctx.enter_context(tc.tile_pool(name="wpool", bufs=1))
        mega_pool = ctx.enter_context(tc.tile_pool(name="mega", bufs=mega_bufs))
        in_pool = ctx.enter_context(tc.tile_pool(name="inp", bufs=io_bufs))
        tmp_pool = ctx.enter_context(tc.tile_pool(name="tmp", bufs=tmp_bufs))
        psum_pool = ctx.enter_context(
            tc.tile_pool(name="ps", bufs=psum_bufs, space="PSUM")
        )
        pin_pool = ctx.enter_context(
            tc.tile_pool(name="pin", bufs=pin_bufs, space="PSUM")
        )
        cm_pool = ctx.enter_context(tc.tile_pool(name="cm", bufs=cm_bufs))

        wt_sb = wpool.tile([128, 768], mybir.dt.float8e4)
        nc.sync.dma_start(out=wt_sb[:], in_=wt[:])
        id_sb = wpool.tile([128, 256], mybir.dt.bfloat16)
        nc.sync.dma_start(out=id_sb[:], in_=idents[:])

        groups = []
        r0 = 0
        while r0 < H:
            g = min(gsize, H - r0)
            groups.append((r0, g))
            r0 += g

        def input_loads(img, first_half_only=False):
            mega = mega_pool.tile([128, MEGA_COLS], mybir.dt.float8e4)

            # whole-image linear load (row per partition) with fp32 -> bf16
            # cast in the DMA (SWDGE)
            xt = in_pool.tile([128, H * 64], mybir.dt.bfloat16)
            rows = 64 if first_half_only else H
            nc.gpsimd.dma_start(
                out=xt[0:rows, :],
                in_=x[img, 0:rows].rearrange("r w c -> r (w c)"),
            )

            # zero pads.  Each slot's col 0 is half-pad: top (odd) is the
            # left-edge zero, bottom holds real data (even@0, written by the
            # ACT below).  Col 64's bottom is the right-edge zero (even@64);
            # its top holds odd@63.  Row slots -1/128 are fully zero.
            nc.gpsimd.memset(mega[:, 0:2], 0.0)
            slots = mega[:, OFF : OFF + 130 * SW].rearrange(
                "p (s c) -> p s c", c=SW
            )
            nc.gpsimd.memset(slots[0:64, :, 0:1], 0.0)
            nc.gpsimd.memset(slots[64:128, :, 64:65], 0.0)
            nc.gpsimd.memset(mega[:, OFF : OFF + SW], 0.0)
            b128 = OFF + 129 * SW
            nc.gpsimd.memset(mega[:, b128 : b128 + SW], 0.0)

            return mega, xt

        def input_edges(img, mega, xt):
            # Edge columns on the PE: odd@63 (pixel 127) and even@0
            # (pixel 0) channel transposes.
            pin = pin_pool.tile([128, 128], mybir.dt.float32)
            nc.tensor.matmul(
                pin[0:64, :],
                xt[:, 127 * 64 : 128 * 64],
                id_sb[:, 0:128],
                start=True,
                stop=True,
            )
            nc.tensor.matmul(
                pin[64:128, :],
                xt[:, 0:64],
                id_sb[:, 0:128],
                start=True,
                stop=True,
            )
            # odd@63 -> top of col 64 of each row slot
            dsl2 = mega[0:64, OFF + SW + 64 : OFF + SW + 65]
            dst2 = bass.AP(
                dsl2.tensor, dsl2.offset, [list(dsl2.ap[0]), [SW, H]]
            )
            ssl2 = pin[0:64, 0:1]
            srcap2 = bass.AP(
                ssl2.tensor, ssl2.offset, [list(ssl2.ap[0]), [1, H]]
            )
            nc.scalar.activation(
                dst2, srcap2, mybir.ActivationFunctionType.Sign
            )
            # even@0 -> bottom of col 0 of each row slot
            dsl3 = mega[64:128, OFF + SW : OFF + SW + 1]
            dst3 = bass.AP(
                dsl3.tensor, dsl3.offset, [list(dsl3.ap[0]), [SW, H]]
            )
            ssl3 = pin[64:128, 0:1]
            srcap3 = bass.AP(
                ssl3.tensor, ssl3.offset, [list(ssl3.ap[0]), [1, H]]
            )
            nc.scalar.activation(
                dst3, srcap3, mybir.ActivationFunctionType.Sign
            )

        def input_xbar_sign_full(img, mega, xt):
            # whole-image variant: a single xbar (one DMA barrier) + one
            # full-width Sign covering all 128 rows.
            tmp = tmp_pool.tile([128, 63 * 128], mybir.dt.bfloat16)
            nc.scalar.dma_start_transpose(
                out=tmp[:].rearrange("p (g f) -> p g f", f=128),
                in_=xt[:, 64 : 64 + 63 * 128],
            )
            dsl = mega[:, OFF + SW + 1 : OFF + SW + 2]
            dst = bass.AP(
                dsl.tensor,
                dsl.offset,
                [list(dsl.ap[0]), [SW, H], [1, 63]],
            )
            ssl = tmp[:, 0:1]
            srcap = bass.AP(
                ssl.tensor,
                ssl.offset,
                [list(ssl.ap[0]), [1, H], [128, 63]],
            )
            nc.scalar.activation(
                dst, srcap, mybir.ActivationFunctionType.Sign
            )

        def input_xbar_sign_half(img, mega, xt, h):
            # 64-row half variant (image 0 warm-up): two smaller xbar +
            # Sign chains let the first conv batches start ~14us earlier.
            HH = 64
            tmp = tmp_pool.tile([128, 63 * HH], mybir.dt.bfloat16)
            nc.scalar.dma_start_transpose(
                out=tmp[:].rearrange("p (g f) -> p g f", f=HH),
                in_=xt[HH * h : HH * h + HH, 64 : 64 + 63 * 128],
            )
            b0 = OFF + (HH * h + 1) * SW + 1
            dsl = mega[:, b0 : b0 + 1]
            dst = bass.AP(
                dsl.tensor,
                dsl.offset,
                [list(dsl.ap[0]), [SW, HH], [1, 63]],
            )
            ssl = tmp[:, 0:1]
            srcap = bass.AP(
                ssl.tensor,
                ssl.offset,
                [list(ssl.ap[0]), [1, HH], [HH, 63]],
            )
            nc.scalar.activation(
                dst, srcap, mybir.ActivationFunctionType.Sign
            )

        def input_warmup_pe(img, mega, xt, t):
            # warm-up path (image 0 only): transpose a 32-row chunk on the
            # otherwise-idle PE via K=32 identity matmuls, 16 pair-blocks
            # per PSUM tile, so the first conv can start ~10us in without
            # waiting for the whole-image xbar chain.
            b0 = OFF + (RC * t + 1) * SW + 1
            for q in range(4):
                pin = pin_pool.tile(
                    [128, 16 * RC], mybir.dt.float32, padded_shape=[128, 512]
                )
                for gg in range(16):
                    g = 16 * q + gg
                    if g >= 63:
                        break
                    nc.tensor.matmul(
                        pin[:, RC * gg : RC * gg + RC],
                        xt[RC * t : RC * t + RC, 64 + 128 * g : 192 + 128 * g],
                        id_sb[RC * t : RC * t + RC, RC * t : RC * t + RC],
                        start=True,
                        stop=True,
                        tile_position=(RC * t, 0),
                    )
                ng = min(16, 63 - 16 * q)
                dsl = mega[:, b0 + 16 * q : b0 + 16 * q + 1]
                dst = bass.AP(
                    dsl.tensor,
                    dsl.offset,
                    [list(dsl.ap[0]), [SW, RC], [1, ng]],
                )
                ssl = pin[:, 0:1]
                srcap = bass.AP(
                    ssl.tensor,
                    ssl.offset,
                    [list(ssl.ap[0]), [1, RC], [RC, ng]],
                )
                nc.scalar.activation(
                    dst, srcap, mybir.ActivationFunctionType.Sign
                )

        GB = gb
        batches = [groups[i : i + GB] for i in range(0, len(groups), GB)]
        if len(batches) >= 2 and len(batches[-1]) < GB // 2:
            batches[-2].extend(batches.pop())

        def compute_batch(img, mega, batch):
            if True:
                rb0 = batch[0][0]
                RB = sum(g for _, g in batch)
                cm = cm_pool.tile([128, RB * 64], mybir.dt.float16)
                cmoff = 0
                for r0, g in batch:
                    N = g * SW
                    ps = psum_pool.tile(
                        [128, N], mybir.dt.float32, padded_shape=[128, 512]
                    )
                    # 6 passes: per dy, aligned (A) + minus-one-shifted (B)
                    mms = []
                    for dy in range(3):
                        ibA = OFF + (r0 + dy) * SW
                        mms.append(nc.tensor.matmul(
                            ps[:, :],
                            wt_sb[:, dy * 256 : dy * 256 + 128],
                            mega[:, ibA : ibA + N],
                            start=(dy == 0),
                            stop=False,
                        ))
                        mms.append(nc.tensor.matmul(
                            ps[:, :],
                            wt_sb[:, dy * 256 + 128 : dy * 256 + 256],
                            mega[:, ibA - 1 : ibA - 1 + N],
                            start=False,
                            stop=(dy == 2),
                        ))
                    for a, b in zip(mms[1:], mms[:-1]):
                        add_dep_helper(
                            a.ins, b.ins, sync=False, reason="psum group order"
                        )

                    # strip pads + cast into the batch's cm tile
                    ps_v = ps[:].rearrange("p (s c) -> p s c", c=SW)[:, :, 1:65]
                    nc.vector.tensor_copy(
                        out=cm[:, cmoff : cmoff + g * 64].rearrange(
                            "p (s c) -> p s c", c=64
                        ),
                        in_=ps_v,
                    )
                    cmoff += g * 64

                # direct channel-major store on the SP HWDGE queue
                # (host unshuffles to NHWC)
                nc.sync.dma_start(
                    out=y[img, :, rb0 * 64 : (rb0 + RB) * 64], in_=cm[:]
                )

        # software-pipelined emission.  The image load is prefetched one
        # iteration ahead (emitted right after the input stage so it lands
        # on the DMA device before the stores); each iteration runs the
        # (tiny) PE edge transposes, the xbar + Sign chain for image it,
        # and the conv batches of image it-1.  Image 0 uses two half-image
        # xbars so the first conv batches start earlier.
        megas = {}
        megas[0] = input_loads(0)
        for it in range(nimg + 1):
            if it < nimg:
                mega, xt = megas[it]
                if it == 0:
                    # warm-up: two half-image xbars let the first conv
                    # batches start earlier
                    input_xbar_sign_half(it, mega, xt, 0)
                    input_xbar_sign_half(it, mega, xt, 1)
                    input_edges(it, mega, xt)
                else:
                    input_edges(it, mega, xt)
                    input_xbar_sign_full(it, mega, xt)
            if it + 1 < nimg:
                megas[it + 1] = input_loads(it + 1)
            for batch in batches:
                if it >= 1:
                    compute_batch(it - 1, megas[it - 1][0], batch)
            if it >= 1:
                del megas[it - 1]

    return nc


def make_wt(w_np):
    """Host-side weight prep: w [3,3,64,64] fp32 -> wt [128, 768] fp8e4.

    Mega column c holds [top k<64: odd@c | bottom k>=64: even@(c+1)].
    Per dy, pass A ([128,128] at cols dy*256, rhs base ibA): column j
    delivers odd@j (top) and even@(j+1) (bottom).  Pass B (at cols
    dy*256+128, rhs base ibA-1): column j delivers odd@(j-1) and even@j.
    M cols 0:64 = even-pixel outputs, 64:128 = odd-pixel outputs.
    """
    ws = np.sign(w_np).astype(np.float32)
    wt = np.zeros((128, 768), np.float32)
    for dy in range(3):
        WA = np.zeros((128, 128), np.float32)
        WA[0:64, 0:64] = ws[dy, 2]      # odd@j -> even-out (dx=2)
        WA[0:64, 64:128] = ws[dy, 1]    # odd@j -> odd-out (dx=1)
        WA[64:128, 64:128] = ws[dy, 2]  # even@(j+1) -> odd-out (dx=2)
        WB = np.zeros((128, 128), np.float32)
        WB[0:64, 0:64] = ws[dy, 0]      # odd@(j-1) -> even-out (dx=0)
        WB[64:128, 0:64] = ws[dy, 1]    # even@j -> even-out (dx=1)
        WB[64:128, 64:128] = ws[dy, 0]  # even@j -> odd-out (dx=0)
        wt[:, dy * 256 : dy * 256 + 128] = WA
        wt[:, dy * 256 + 128 : dy * 256 + 256] = WB
    fp8 = mybir.dt.np(mybir.dt.float8e4)
    return wt.astype(fp8)


def make_idents():
    """[I | Iz] [128, 256] bf16: Iz[k, n] = (k == n+1) & (n % 4 != 3)."""
    idents = np.zeros((128, 256), np.float32)
    idents[:, 0:128] = np.eye(128)
    for n in range(127):
        if n % 4 != 3:
            idents[n + 1, 128 + n] = 1.0
    return idents.astype(ml_dtypes.bfloat16)


_NC_CACHE = {}


def get_nc():
    if "nc" not in _NC_CACHE:
        _NC_CACHE["nc"] = build_nc()
    return _NC_CACHE["nc"]


def kernel(x, w):
    from concourse.bass_utils import run_bass_kernel_spmd

    x = np.asarray(x, dtype=np.float32)
    w = np.asarray(w, dtype=np.float32)
    assert x.shape == (N_CORES * NIMG, H, W, C) and w.shape == (3, 3, C, C)

    wt = make_wt(w)
    idents = make_idents()
    nc = get_nc()
    in_maps = [
        {
            "x": np.ascontiguousarray(x[c * NIMG : (c + 1) * NIMG]),
            "wt": wt,
            "idents": idents,
        }
        for c in range(N_CORES)
    ]
    res = run_bass_kernel_spmd(nc, in_maps, list(range(N_CORES)))
    outs = []
    for c in range(N_CORES):
        ycm = np.asarray(res.results[c]["y"])  # [nimg, 128, H*64] fp16
        o = (
            ycm.reshape(NIMG, 2, 64, H, 64)  # (img, par, co, r, j)
            .transpose(0, 3, 4, 1, 2)        # (img, r, j, par, co)
            .reshape(NIMG, H, W, C)
            .astype(np.float32)
        )
        outs.append(o)
    return np.concatenate(outs, axis=0)
